# revision 19
# baseline (speedup 1.0000x reference)
"""Trainium2 Bass kernel for the DeformationGraph problem.

Math: per batch b and vertex v,
    out[b,v,k] = sum_c W[v,c] * ( sum_d (X[b,v,d]-center[b,c,d]) * R[b,c,k,d]
                                  + center[b,c,k] + V_nodes[b,c,k] )
factors into a vertex-independent per-node affine map:
    t[b,c,k]   = center[b,c,k] + V_nodes[b,c,k] - sum_d center[b,c,d]*R[b,c,k,d]
    out[b,v,k] = sum_d X[b,v,d] * (W @ R[..,k,d])[v]  +  (W @ t[..,k])[v]
i.e. one (V,C)@(C,48) matmul Y = W @ G, then a per-vertex contraction of Y
with [X,1].  W/X/out are sharded over the vertex dim across 8 cores.

Precision: rel-err budget is 2e-2.  W ships as fp8 e3m4 (x64 host scale,
undone by a /64 baked into the X table) - W's quantization alone measures
1.42e-2 end-to-end; everything else is fp16 so it adds nothing.

Per-core pipeline (vertex shard padded to 6272 = 6*1024 + 128):
  - PE, per 1024-vertex pair (G-column layout j = k*16 + d*4 + b, d==3 =
    translation, cols 48:64 zero):
      A_e (K=128, y rows 0:64), A_o (K=128, rows 64:128), and the K=32
      B-part as ONE block-diag [64,128] matmul over vertex-pair columns
      (whb[(h*32+c), 512p+j] = W_B[c, 1024p+512h+j]) accumulating into
      the same y tile.
  - DVE: one [128,512] tensor_mul  s = y * xd2  (fp16 out to SBUF).
  - PE: the 4-way d-reduction as a 0/1 matmul r[24,n] = RED^T @ s,
    lagged behind so the PE never waits on the DVE.
  - ACT: r (PSUM) -> ro (SBUF, fp16) copies; two merged out DMAs on the
    sync queue.
DMA plan: ~1.8MB/core HBM over three queues (sync/scalar HWDGE + gpsimd
SWDGE).  Early items per queue are the ones gating pair 0.  Warmup
matmuls use memset-zero weights so the PE DVFS ramps during the DMA
wait with no data dependency.
"""

import numpy as np
import ml_dtypes

import concourse.mybir as mybir
import concourse.tile as tile
from concourse import bacc
from concourse.bass_utils import run_bass_kernel_spmd

B, V, C = 4, 50000, 160
N_CORES = 8
VS = V // N_CORES            # 6250 vertices per core
VSP = 6272                   # padded shard: 6 pairs of 1024 + 128 tail
NPAIR = 6
PC = 3200                    # pair-col space: 6*512 + 128
F32 = mybir.dt.float32
F16 = mybir.dt.float16
F8 = mybir.dt.float8e3
NPF16 = np.float16
NPF8 = ml_dtypes.float8_e3m4

WSC = 64.0                   # host scale on W (fp8), undone via X table
CW = 224                     # cst tensor cols: ghA 64 | RED 24 | pad | ghB2 128
# wha chunks (vertex cols), one per 2-pair group, all on the scalar
# HWDGE queue; 2KB-per-partition descriptors keep the SDMA engines at
# full rate.
WCH = [(0, 2048), (2048, 4096), (4096, VSP)]
BCH = [(0, PC)]                # whb: one early DMA on gpsimd
XCH = [(0, 1024), (1024, 2048), (2048, PC)]  # xd2 chunks
N_WARM = 27                    # N=128 zero-weight PE ramp matmuls (PE has
                               # p-states: 2.4GHz only after 3us continuous,
                               # and any idle gap resets the ramp)


def _locate(tiles, chunks, g0, width):
    for t, (c0, c1) in zip(tiles, chunks):
        if c0 <= g0 and g0 + width <= c1:
            return t, slice(g0 - c0, g0 - c0 + width)
    raise AssertionError(f"col range {g0}+{width} crosses chunk boundary")


def _build_bass():
    nc = bacc.Bacc()

    wha_d = nc.dram_tensor("wha", [128, VSP], F8, kind="ExternalInput")
    whb_d = nc.dram_tensor("whb", [64, PC], F8, kind="ExternalInput")
    cst_d = nc.dram_tensor("cst", [128, CW], F16, kind="ExternalInput")
    xc_d = nc.dram_tensor("xc", [128, PC], F16, kind="ExternalInput")
    outT = nc.dram_tensor("outT", [24, PC], F16, kind="ExternalOutput")

    with tile.TileContext(nc) as tc:
        with (
            tc.tile_pool(name="cpool", bufs=1) as cpool,
            tc.tile_pool(name="spool", bufs=4) as spool,
            tc.tile_pool(name="ypool", bufs=4, space="PSUM") as ypool,
            tc.tile_pool(name="rpool", bufs=3, space="PSUM") as rpool,
        ):
            # zero-weight PE ramp: no input dependency, starts right after
            # the preamble and keeps the DVFS governor fed until real
            # work arrives
            wsc = cpool.tile([128, 192], F16, tag="wsc")
            nc.vector.memset(wsc[:], 0.0)
            ywarm = ypool.tile([64, 128], F32, tag="ywarm", bufs=1)
            for w in range(N_WARM):
                nc.tensor.matmul(ywarm[:], wsc[:, 0:64], wsc[:, 64:192],
                                 start=(w == 0), stop=(w == N_WARM - 1),
                                 skip_group_check=True)

            # --- input DMAs, per-queue in need-order ---
            # sync:   cst | xc0 | xcm  (later: the out DMAs)
            # scalar: wha0 | wha1 | wha2 (then the r->ro copies)
            # gpsimd: whb | xc1
            cst = cpool.tile([128, CW], F16, tag="cst")
            nc.sync.dma_start(out=cst[:], in_=cst_d[:])
            whb = cpool.tile([64, PC], F8, tag="whb")
            nc.gpsimd.dma_start(out=whb[:], in_=whb_d[:])
            whb_t = [whb]

            wha_t = []
            for i, (c0, c1) in enumerate(WCH):
                wt = cpool.tile([128, c1 - c0], F8, tag=f"wha{i}",
                                name=f"wha{i}")
                nc.scalar.dma_start(out=wt[:], in_=wha_d[:, c0:c1])
                wha_t.append(wt)
                if i == 0:
                    xc0 = cpool.tile([128, 1024], F16, tag="xc0")
                    nc.sync.dma_start(out=xc0[:], in_=xc_d[:, 0:1024])

            xcm = cpool.tile([128, 1024], F16, tag="xcm")
            nc.sync.dma_start(out=xcm[:], in_=xc_d[:, 1024:2048])
            xc1 = cpool.tile([128, PC - 2048], F16, tag="xc1")
            nc.gpsimd.dma_start(out=xc1[:], in_=xc_d[:, 2048:PC])
            xc_t = [xc0, xcm, xc1]

            ghA = cst[:, 0:64]
            RED24 = cst[:, 64:88]
            ghB2 = cst[0:64, 96:224]  # block-diag [[G_B,0],[0,G_B]]

            ro = cpool.tile([24, PC], F16, tag="ro")

            # 2-pair groups: A(p)h0 A(p)h1 A(p+1)h0 A(p+1)h1 share one ghA
            # load; B(p) B(p+1) share one ghB2 load; the REDs of the
            # previous group (one RED24 load) slot in after, so the PE
            # stream has at most 3 weight switches per group and no waits
            # on the DVE.
            def emit_A(p):
                y = ypool.tile([128, 512], F32, tag="y", bufs=4)
                for h in range(2):
                    g0 = 1024 * p + 512 * h
                    wa, sa = _locate(wha_t, WCH, g0, 512)
                    nc.tensor.matmul(y[64 * h:64 * h + 64, :], ghA,
                                     wa[:, sa], start=True, stop=False,
                                     skip_group_check=True)
                return y

            def emit_B(p, y):
                wb, sb = _locate(whb_t, BCH, 512 * p, 512)
                nc.tensor.matmul(y[:], ghB2, wb[:, sb],
                                 start=False, stop=True,
                                 skip_group_check=True)

            def emit_mul(p, y):
                xt, sx = _locate(xc_t, XCH, 512 * p, 512)
                s = spool.tile([128, 512], F16, tag="s")
                nc.vector.tensor_mul(out=s[:], in0=y[:], in1=xt[:, sx])
                return s

            def emit_red(p, s, copy_eng="scalar"):
                r = rpool.tile([24, 512], F32, tag="r", bufs=3)
                nc.tensor.matmul(r[:], RED24, s[:], start=True, stop=True,
                                 skip_group_check=True)
                csl = slice(512 * p, 512 * p + 512)
                if copy_eng == "scalar":
                    nc.scalar.copy(out=ro[:, csl], in_=r[:])
                else:
                    nc.vector.tensor_copy(out=ro[:, csl], in_=r[:])

            s_tiles = {}

            def emit_group(pa, pb, reds):
                ya = emit_A(pa)
                yb = emit_A(pb)
                emit_B(pa, ya)
                emit_B(pb, yb)
                s_tiles[pa] = emit_mul(pa, ya)
                s_tiles[pb] = emit_mul(pb, yb)
                for rp in reds:
                    emit_red(rp, s_tiles.pop(rp))

            def pe_filler(n):
                # zero-weight matmuls bridge a DMA-gated gap in the PE
                # stream: an idle >~1us drops the PE p-state (2.4GHz ->
                # 1.2GHz) for the rest of the kernel, active waiting
                # does not
                for w in range(n):
                    nc.tensor.matmul(ywarm[:], wsc[:, 0:64], wsc[:, 64:192],
                                     start=True, stop=True,
                                     skip_group_check=True)

            emit_group(0, 1, [])
            emit_group(2, 3, [0, 1])
            pe_filler(N_FILL)
            emit_group(4, 5, [2, 3])
            # ro[0:2048] complete once RED3's copy lands; overlap its DMA
            # with the remaining PE/DVE tail
            nc.sync.dma_start(out=outT[:, 0:2048], in_=ro[:, 0:2048])

            # 128-vertex tail (single half) keeps the PE busy while the
            # DVE catches up on pairs 4/5
            yt = ypool.tile([128, 512], F32, tag="y", bufs=4)
            wa, sa = _locate(wha_t, WCH, 6144, 128)
            wb, sb = _locate(whb_t, BCH, 3072, 128)
            xt, sx = _locate(xc_t, XCH, 3072, 128)
            nc.tensor.matmul(yt[0:64, 0:128], ghA, wa[:, sa],
                             start=True, stop=False, skip_group_check=True)
            nc.tensor.matmul(yt[0:64, 0:128], ghB2[:, 0:64], wb[:, sb],
                             start=False, stop=True, skip_group_check=True)
            st = spool.tile([128, 512], F16, tag="s")
            nc.vector.tensor_mul(out=st[0:64, 0:128], in0=yt[0:64, 0:128],
                                 in1=xt[0:64, sx])

            emit_red(4, s_tiles.pop(4))
            emit_red(5, s_tiles.pop(5), copy_eng="vector")
            rt = rpool.tile([24, 512], F32, tag="r", bufs=3)
            nc.tensor.matmul(rt[:, 0:128], cst[0:64, 64:88],
                             st[0:64, 0:128],
                             start=True, stop=True, skip_group_check=True)
            nc.scalar.copy(out=ro[:, 3072:3200], in_=rt[:, 0:128])
            nc.sync.dma_start(out=outT[:, 2048:3200], in_=ro[:, 2048:3200])
    nc.finalize()
    return nc


_NC_CACHE = None


def _get_nc():
    global _NC_CACHE
    if _NC_CACHE is None:
        _NC_CACHE = _build_bass()
    return _NC_CACHE


def _host_prep(X, V_nodes, rot6d_nodes, W_nodes, idx_nn_to_nodes):
    """Small per-node math (B*C=640 rows) + shard/layout of the big tensors."""
    X = np.asarray(X, np.float32)
    Vn = np.asarray(V_nodes, np.float32)
    d6 = np.asarray(rot6d_nodes, np.float32)
    W = np.asarray(W_nodes, np.float32)
    idx = np.asarray(idx_nn_to_nodes).astype(np.int64)

    a1, a2 = d6[..., :3], d6[..., 3:]
    eps = np.float32(1e-8)
    n1 = np.sqrt(np.sum(a1 * a1, -1, keepdims=True, dtype=np.float32))
    b1 = a1 / np.maximum(n1, eps)
    dot = np.sum(b1 * a2, -1, keepdims=True, dtype=np.float32)
    a2p = a2 - dot * b1
    n2 = np.sqrt(np.sum(a2p * a2p, -1, keepdims=True, dtype=np.float32))
    b2 = a2p / np.maximum(n2, eps)
    b3 = np.cross(b1, b2)
    R = np.stack([b1, b2, b3], axis=-2).astype(np.float32)  # (B,C,3,3) [b,c,k,d]

    center = X[:, idx, :]                                   # (B,C,3)
    t = (center + Vn - np.einsum('bcd,bckd->bck', center, R)).astype(np.float32)

    # G columns at j = k*16 + d*4 + b (d==3 = translation); cols 48:64 zero
    Gv = np.zeros((C, 4, 4, 4), np.float32)
    Gv[:, 0:3, 0:3, :] = np.transpose(R, (1, 2, 3, 0))
    Gv[:, 0:3, 3, :] = np.transpose(t, (1, 2, 0))
    G = Gv.reshape(C, 64)

    RED = np.zeros((2, 4, 4, 4, 24), np.float32)
    for h in range(2):
        for k in range(3):
            for b in range(B):
                RED[h, k, :, b, h * 12 + k * 4 + b] = 1.0
    RED = RED.reshape(128, 24)

    cst = np.zeros((128, CW), NPF16)
    cst[:, 0:64] = G[0:128].astype(NPF16)
    cst[:, 64:88] = RED.astype(NPF16)
    gB = G[128:160].astype(NPF16)              # [32, 64]
    cst[0:32, 96:160] = gB                     # even-half block
    cst[32:64, 160:224] = gB                   # odd-half block

    Wb = (W * np.float32(WSC)).astype(NPF8)
    inv = np.float32(1.0 / WSC)
    in_maps = []
    for i in range(N_CORES):
        vsl = slice(i * VS, (i + 1) * VS)
        wt = np.zeros((160, VSP), NPF8)
        wt[:, :VS] = Wb[vsl].T
        wha = np.ascontiguousarray(wt[0:128])
        # B part in vertex-pair-column layout: whb[(h*32+c), 512p+j] =
        # W_B[c, 1024p + 512h + j]; tail (cols 3072:3200) even-half only
        bp = wt[128:160]                       # [32, VSP]
        whb = np.zeros((64, PC), NPF8)
        whb[:, 0:3072] = bp[:, :6144].reshape(32, NPAIR, 2, 512).transpose(
            2, 0, 1, 3).reshape(64, 3072)
        whb[0:32, 3072:3200] = bp[:, 6144:6272]

        Xs = np.zeros((B, VSP, 3), np.float32)
        Xs[:, :VS] = X[:, vsl, :] * inv
        xc = np.zeros((2, 4, 4, PC), np.float32)        # [h, d, b, col]
        main = Xs[:, :6144].reshape(B, NPAIR, 2, 512, 3)
        xc[:, 0:3, :, 0:3072] = np.transpose(
            main, (2, 4, 0, 1, 3)).reshape(2, 3, B, 3072)
        xc[:, 3, :, 0:3072] = inv
        xc[0, 0:3, :, 3072:3200] = np.transpose(Xs[:, 6144:6272], (2, 0, 1))
        xc[0, 3, :, 3072:3200] = inv
        xc = xc.reshape(2, 16, PC)
        # fully inflated xd2: 4 k-copies per half -> [128, PC]
        xcc = np.concatenate([xc[0]] * 4 + [xc[1]] * 4, 0).astype(NPF16)

        in_maps.append({"wha": wha, "whb": whb, "cst": cst, "xc": xcc})
    return in_maps


def _gather(results):
    out = np.empty((B, V, 3), np.float32)
    for i, res in enumerate(results):
        o = res["outT"].astype(np.float32).reshape(2, 3, 4, PC)  # [h,k,b,col]
        om = o[:, :, :, 0:3072].reshape(2, 3, 4, NPAIR, 512)
        block = np.empty((B, VSP, 3), np.float32)
        block[:, :6144] = np.transpose(om, (2, 3, 0, 4, 1)).reshape(B, 6144, 3)
        block[:, 6144:6272] = np.transpose(o[0, :, :, 3072:3200], (1, 2, 0))
        out[:, i * VS:(i + 1) * VS] = block[:, :VS]
    return out


def kernel(X, V_nodes, rot6d_nodes, W_nodes, idx_nn_to_nodes, **run_kwargs):
    in_maps = _host_prep(X, V_nodes, rot6d_nodes, W_nodes, idx_nn_to_nodes)
    res = run_bass_kernel_spmd(_get_nc(), in_maps,
                               core_ids=list(range(N_CORES)), **run_kwargs)
    out = _gather(res.results)
    kernel.last_run = res
    return out


# revision 21
# speedup vs baseline: 1.0894x; 1.0894x over previous
"""Trainium2 Bass kernel for the DeformationGraph problem.

Math: per batch b and vertex v,
    out[b,v,k] = sum_c W[v,c] * ( sum_d (X[b,v,d]-center[b,c,d]) * R[b,c,k,d]
                                  + center[b,c,k] + V_nodes[b,c,k] )
factors into a vertex-independent per-node affine map:
    t[b,c,k]   = center[b,c,k] + V_nodes[b,c,k] - sum_d center[b,c,d]*R[b,c,k,d]
    out[b,v,k] = sum_d X[b,v,d] * (W @ R[..,k,d])[v]  +  (W @ t[..,k])[v]
i.e. one (V,C)@(C,48) matmul Y = W @ G, then a per-vertex contraction of Y
with [X,1].  W/X/out are sharded over the vertex dim across 8 cores.

Precision: rel-err budget is 2e-2.  W ships as fp8 e3m4 (x64 host scale,
undone by a /64 baked into the X table) - W's quantization alone measures
1.42e-2 end-to-end; everything else is fp16 so it adds nothing.  The PE
takes mixed fp16(stationary) x fp8e3(moving) matmuls natively.

Per-core pipeline (vertex shard padded to 6272 = 6*1024 + 128):
  - PE, per 1024-vertex pair (G-column layout j = k*16 + d*4 + b, d==3 =
    translation, cols 48:64 zero), emitted in 2-pair GROUPS so one ghA
    LDWEIGHTS serves 4 A-matmuls and one ghB2 serves 2 B-matmuls:
      A_e (K=128, y rows 0:64), A_o (K=128, rows 64:128), and the K=32
      B-part as ONE block-diag [64,128] matmul over vertex-pair columns
      (whb[(h*32+c), 512p+j] = W_B[c, 1024p+512h+j]) accumulating into
      the same y tile.
  - DVE: one [128,512] tensor_mul  s = y * xd2  (fp16 out to SBUF).
  - PE: the 4-way d-reduction as a 0/1 matmul r[24,n] = RED^T @ s,
    lagged one group behind so the PE never waits on the DVE.
  - ACT/DVE: r (PSUM) -> ro (SBUF fp16) copies; two out DMAs on sync.

Clock management (the big lever): the PE has p-states - 1.2GHz default,
2.4GHz only after ~3us of continuous execution, and an idle gap >~1us
drops it back for the rest of the kernel (a 512-col matmul is ~630ns at
MID vs ~380ns at MAX).  Zero-weight warmup matmuls ramp the clock while
the first DMAs land, and zero-weight FILLER matmuls bridge the
DMA-gated gaps between pair groups so the achieved p-state survives.

DMA plan (~1.8MB/core over three queues, each DMA >=2KB per-partition
descriptors - 1KB descriptors run ~2-3x slower):
  scalar HWDGE: wha [0:2048],[2048:4096],[4096:6272] (fp8, one per group)
  sync HWDGE:   cst | xc0 | xcm   then the two output DMAs
  gpsimd SWDGE: whb (one DMA) | xc1
Aggregate sustains ~300GB/s; the body is DMA-bound, so group g's inputs
land just-in-time and the fillers absorb the +-1us arrival jitter.
"""

import numpy as np
import ml_dtypes

import concourse.mybir as mybir
import concourse.tile as tile
from concourse import bacc
from concourse.bass_utils import run_bass_kernel_spmd

B, V, C = 4, 50000, 160
N_CORES = 8
VS = V // N_CORES            # 6250 vertices per core
VSP = 6272                   # padded shard: 6 pairs of 1024 + 128 tail
NPAIR = 6
PC = 3200                    # pair-col space: 6*512 + 128
F32 = mybir.dt.float32
F16 = mybir.dt.float16
F8 = mybir.dt.float8e3
NPF16 = np.float16
NPF8 = ml_dtypes.float8_e3m4

WSC = 64.0                   # host scale on W (fp8), undone via X table
CW = 224                     # cst tensor cols: ghA 64 | RED 24 | pad | ghB2 128
# wha chunks (vertex cols), one per 2-pair group, all on the scalar
# HWDGE queue; 2KB-per-partition descriptors keep the SDMA engines at
# full rate.
WCH = [(0, 2048), (2048, 4096), (4096, VSP)]
BCH = [(0, PC)]                # whb: one early DMA on gpsimd
XCH = [(0, 1024), (1024, 2048), (2048, PC)]  # xd2 chunks
N_WARM = 27                    # N=128 zero-weight PE ramp matmuls (PE has
                               # p-states: 2.4GHz only after 3us continuous,
                               # and any idle gap resets the ramp)


def _locate(tiles, chunks, g0, width):
    for t, (c0, c1) in zip(tiles, chunks):
        if c0 <= g0 and g0 + width <= c1:
            return t, slice(g0 - c0, g0 - c0 + width)
    raise AssertionError(f"col range {g0}+{width} crosses chunk boundary")


def _build_bass():
    nc = bacc.Bacc()

    wha_d = nc.dram_tensor("wha", [128, VSP], F8, kind="ExternalInput")
    whb_d = nc.dram_tensor("whb", [64, PC], F8, kind="ExternalInput")
    cst_d = nc.dram_tensor("cst", [128, CW], F16, kind="ExternalInput")
    xc_d = nc.dram_tensor("xc", [128, PC], F16, kind="ExternalInput")
    outT = nc.dram_tensor("outT", [24, PC], F16, kind="ExternalOutput")

    with tile.TileContext(nc) as tc:
        with (
            tc.tile_pool(name="cpool", bufs=1) as cpool,
            tc.tile_pool(name="spool", bufs=4) as spool,
            tc.tile_pool(name="ypool", bufs=4, space="PSUM") as ypool,
            tc.tile_pool(name="rpool", bufs=3, space="PSUM") as rpool,
        ):
            # zero-weight PE ramp: no input dependency, starts right after
            # the preamble and keeps the DVFS governor fed until real
            # work arrives
            wsc = cpool.tile([128, 192], F16, tag="wsc")
            nc.vector.memset(wsc[:], 0.0)
            ywarm = ypool.tile([64, 128], F32, tag="ywarm", bufs=1)
            for w in range(N_WARM):
                nc.tensor.matmul(ywarm[:], wsc[:, 0:64], wsc[:, 64:192],
                                 start=(w == 0), stop=(w == N_WARM - 1),
                                 skip_group_check=True)

            # --- input DMAs, per-queue in need-order ---
            # sync:   cst | xc0 | xcm  (later: the out DMAs)
            # scalar: wha0 | wha1 | wha2 (then the r->ro copies)
            # gpsimd: whb | xc1
            cst = cpool.tile([128, CW], F16, tag="cst")
            nc.sync.dma_start(out=cst[:], in_=cst_d[:])
            whb = cpool.tile([64, PC], F8, tag="whb")
            nc.gpsimd.dma_start(out=whb[:], in_=whb_d[:])
            whb_t = [whb]

            wha_t = []
            for i, (c0, c1) in enumerate(WCH):
                wt = cpool.tile([128, c1 - c0], F8, tag=f"wha{i}",
                                name=f"wha{i}")
                nc.scalar.dma_start(out=wt[:], in_=wha_d[:, c0:c1])
                wha_t.append(wt)
                if i == 0:
                    xc0 = cpool.tile([128, 1024], F16, tag="xc0")
                    nc.sync.dma_start(out=xc0[:], in_=xc_d[:, 0:1024])

            xcm = cpool.tile([128, 1024], F16, tag="xcm")
            nc.sync.dma_start(out=xcm[:], in_=xc_d[:, 1024:2048])
            xc1 = cpool.tile([128, PC - 2048], F16, tag="xc1")
            nc.gpsimd.dma_start(out=xc1[:], in_=xc_d[:, 2048:PC])
            xc_t = [xc0, xcm, xc1]

            ghA = cst[:, 0:64]
            RED24 = cst[:, 64:88]
            ghB2 = cst[0:64, 96:224]  # block-diag [[G_B,0],[0,G_B]]

            ro = cpool.tile([24, PC], F16, tag="ro")

            # 2-pair groups: A(p)h0 A(p)h1 A(p+1)h0 A(p+1)h1 share one ghA
            # load; B(p) B(p+1) share one ghB2 load; the REDs of the
            # previous group (one RED24 load) slot in after, so the PE
            # stream has at most 3 weight switches per group and no waits
            # on the DVE.
            def emit_A(p):
                y = ypool.tile([128, 512], F32, tag="y", bufs=4)
                for h in range(2):
                    g0 = 1024 * p + 512 * h
                    wa, sa = _locate(wha_t, WCH, g0, 512)
                    nc.tensor.matmul(y[64 * h:64 * h + 64, :], ghA,
                                     wa[:, sa], start=True, stop=False,
                                     skip_group_check=True)
                return y

            def emit_B(p, y):
                wb, sb = _locate(whb_t, BCH, 512 * p, 512)
                nc.tensor.matmul(y[:], ghB2, wb[:, sb],
                                 start=False, stop=True,
                                 skip_group_check=True)

            def emit_mul(p, y):
                xt, sx = _locate(xc_t, XCH, 512 * p, 512)
                s = spool.tile([128, 512], F16, tag="s")
                nc.vector.tensor_mul(out=s[:], in0=y[:], in1=xt[:, sx])
                return s

            def emit_red(p, s, copy_eng="scalar"):
                r = rpool.tile([24, 512], F32, tag="r", bufs=3)
                nc.tensor.matmul(r[:], RED24, s[:], start=True, stop=True,
                                 skip_group_check=True)
                csl = slice(512 * p, 512 * p + 512)
                if copy_eng == "scalar":
                    nc.scalar.copy(out=ro[:, csl], in_=r[:])
                else:
                    nc.vector.tensor_copy(out=ro[:, csl], in_=r[:])

            s_tiles = {}

            def emit_group(pa, pb, reds):
                ya = emit_A(pa)
                yb = emit_A(pb)
                emit_B(pa, ya)
                emit_B(pb, yb)
                s_tiles[pa] = emit_mul(pa, ya)
                s_tiles[pb] = emit_mul(pb, yb)
                for rp in reds:
                    emit_red(rp, s_tiles.pop(rp))

            def pe_filler(n):
                # zero-weight matmuls bridge a DMA-gated gap in the PE
                # stream: an idle >~1us drops the PE p-state (2.4GHz ->
                # 1.2GHz) for the rest of the kernel, active waiting
                # does not
                for w in range(n):
                    nc.tensor.matmul(ywarm[:], wsc[:, 0:64], wsc[:, 64:192],
                                     start=True, stop=True,
                                     skip_group_check=True)

            emit_group(0, 1, [])
            pe_filler(4)
            emit_group(2, 3, [0, 1])
            pe_filler(N_FILL)
            emit_group(4, 5, [2, 3])
            # ro[0:2048] complete once RED3's copy lands; overlap its DMA
            # with the remaining PE/DVE tail
            nc.sync.dma_start(out=outT[:, 0:2048], in_=ro[:, 0:2048])

            # 128-vertex tail (single half) keeps the PE busy while the
            # DVE catches up on pairs 4/5
            yt = ypool.tile([128, 512], F32, tag="y", bufs=4)
            wa, sa = _locate(wha_t, WCH, 6144, 128)
            wb, sb = _locate(whb_t, BCH, 3072, 128)
            xt, sx = _locate(xc_t, XCH, 3072, 128)
            nc.tensor.matmul(yt[0:64, 0:128], ghA, wa[:, sa],
                             start=True, stop=False, skip_group_check=True)
            nc.tensor.matmul(yt[0:64, 0:128], ghB2[:, 0:64], wb[:, sb],
                             start=False, stop=True, skip_group_check=True)
            st = spool.tile([128, 512], F16, tag="s")
            nc.vector.tensor_mul(out=st[0:64, 0:128], in0=yt[0:64, 0:128],
                                 in1=xt[0:64, sx])

            emit_red(4, s_tiles.pop(4))
            emit_red(5, s_tiles.pop(5), copy_eng="vector")
            rt = rpool.tile([24, 512], F32, tag="r", bufs=3)
            nc.tensor.matmul(rt[:, 0:128], cst[0:64, 64:88],
                             st[0:64, 0:128],
                             start=True, stop=True, skip_group_check=True)
            nc.scalar.copy(out=ro[:, 3072:3200], in_=rt[:, 0:128])
            nc.sync.dma_start(out=outT[:, 2048:3200], in_=ro[:, 2048:3200])
    nc.finalize()
    return nc


_NC_CACHE = None


def _get_nc():
    global _NC_CACHE
    if _NC_CACHE is None:
        _NC_CACHE = _build_bass()
    return _NC_CACHE


def _host_prep(X, V_nodes, rot6d_nodes, W_nodes, idx_nn_to_nodes):
    """Small per-node math (B*C=640 rows) + shard/layout of the big tensors."""
    X = np.asarray(X, np.float32)
    Vn = np.asarray(V_nodes, np.float32)
    d6 = np.asarray(rot6d_nodes, np.float32)
    W = np.asarray(W_nodes, np.float32)
    idx = np.asarray(idx_nn_to_nodes).astype(np.int64)

    a1, a2 = d6[..., :3], d6[..., 3:]
    eps = np.float32(1e-8)
    n1 = np.sqrt(np.sum(a1 * a1, -1, keepdims=True, dtype=np.float32))
    b1 = a1 / np.maximum(n1, eps)
    dot = np.sum(b1 * a2, -1, keepdims=True, dtype=np.float32)
    a2p = a2 - dot * b1
    n2 = np.sqrt(np.sum(a2p * a2p, -1, keepdims=True, dtype=np.float32))
    b2 = a2p / np.maximum(n2, eps)
    b3 = np.cross(b1, b2)
    R = np.stack([b1, b2, b3], axis=-2).astype(np.float32)  # (B,C,3,3) [b,c,k,d]

    center = X[:, idx, :]                                   # (B,C,3)
    t = (center + Vn - np.einsum('bcd,bckd->bck', center, R)).astype(np.float32)

    # G columns at j = k*16 + d*4 + b (d==3 = translation); cols 48:64 zero
    Gv = np.zeros((C, 4, 4, 4), np.float32)
    Gv[:, 0:3, 0:3, :] = np.transpose(R, (1, 2, 3, 0))
    Gv[:, 0:3, 3, :] = np.transpose(t, (1, 2, 0))
    G = Gv.reshape(C, 64)

    RED = np.zeros((2, 4, 4, 4, 24), np.float32)
    for h in range(2):
        for k in range(3):
            for b in range(B):
                RED[h, k, :, b, h * 12 + k * 4 + b] = 1.0
    RED = RED.reshape(128, 24)

    cst = np.zeros((128, CW), NPF16)
    cst[:, 0:64] = G[0:128].astype(NPF16)
    cst[:, 64:88] = RED.astype(NPF16)
    gB = G[128:160].astype(NPF16)              # [32, 64]
    cst[0:32, 96:160] = gB                     # even-half block
    cst[32:64, 160:224] = gB                   # odd-half block

    Wb = (W * np.float32(WSC)).astype(NPF8)
    inv = np.float32(1.0 / WSC)
    in_maps = []
    for i in range(N_CORES):
        vsl = slice(i * VS, (i + 1) * VS)
        wt = np.zeros((160, VSP), NPF8)
        wt[:, :VS] = Wb[vsl].T
        wha = np.ascontiguousarray(wt[0:128])
        # B part in vertex-pair-column layout: whb[(h*32+c), 512p+j] =
        # W_B[c, 1024p + 512h + j]; tail (cols 3072:3200) even-half only
        bp = wt[128:160]                       # [32, VSP]
        whb = np.zeros((64, PC), NPF8)
        whb[:, 0:3072] = bp[:, :6144].reshape(32, NPAIR, 2, 512).transpose(
            2, 0, 1, 3).reshape(64, 3072)
        whb[0:32, 3072:3200] = bp[:, 6144:6272]

        Xs = np.zeros((B, VSP, 3), np.float32)
        Xs[:, :VS] = X[:, vsl, :] * inv
        xc = np.zeros((2, 4, 4, PC), np.float32)        # [h, d, b, col]
        main = Xs[:, :6144].reshape(B, NPAIR, 2, 512, 3)
        xc[:, 0:3, :, 0:3072] = np.transpose(
            main, (2, 4, 0, 1, 3)).reshape(2, 3, B, 3072)
        xc[:, 3, :, 0:3072] = inv
        xc[0, 0:3, :, 3072:3200] = np.transpose(Xs[:, 6144:6272], (2, 0, 1))
        xc[0, 3, :, 3072:3200] = inv
        xc = xc.reshape(2, 16, PC)
        # fully inflated xd2: 4 k-copies per half -> [128, PC]
        xcc = np.concatenate([xc[0]] * 4 + [xc[1]] * 4, 0).astype(NPF16)

        in_maps.append({"wha": wha, "whb": whb, "cst": cst, "xc": xcc})
    return in_maps


def _gather(results):
    out = np.empty((B, V, 3), np.float32)
    for i, res in enumerate(results):
        o = res["outT"].astype(np.float32).reshape(2, 3, 4, PC)  # [h,k,b,col]
        om = o[:, :, :, 0:3072].reshape(2, 3, 4, NPAIR, 512)
        block = np.empty((B, VSP, 3), np.float32)
        block[:, :6144] = np.transpose(om, (2, 3, 0, 4, 1)).reshape(B, 6144, 3)
        block[:, 6144:6272] = np.transpose(o[0, :, :, 3072:3200], (1, 2, 0))
        out[:, i * VS:(i + 1) * VS] = block[:, :VS]
    return out


def kernel(X, V_nodes, rot6d_nodes, W_nodes, idx_nn_to_nodes, **run_kwargs):
    in_maps = _host_prep(X, V_nodes, rot6d_nodes, W_nodes, idx_nn_to_nodes)
    res = run_bass_kernel_spmd(_get_nc(), in_maps,
                               core_ids=list(range(N_CORES)), **run_kwargs)
    out = _gather(res.results)
    kernel.last_run = res
    return out
